# revision 22
# baseline (speedup 1.0000x reference)
"""Distributed gathered-row matvec kernel for nn_CubicalModel_ISM.

Reference computes Xp = I @ p, Yp = J @ p (I, J: [784, 50000]) and then
gathers 100 (with repeats) elements from each 28x28 reshape. Only the
gathered rows matter, so the kernel:

  1. Host: dedupes the gather rows -> u1 (rows of I), u2 (rows of J),
     NR = |u1| + |u2| (~188 of the 1568 total rows). Builds
     A = concat(I[u1], J[u2]) : [NR, 50000] and computes only A @ p.
  2. Rounds A and p to bf16 (single plane). The bf16 quantization error
     of a 50k-term dot product concentrates around 3e-3 relative --
     far inside the 2e-2 gate -- while halving HBM traffic.
  3. Shards the contraction dim across 8 cores (6272 = 49*128 per core,
     zero padded). Per core a single DRAM stream [128, 49 + 49*NR] bf16
     carries the p chunk (first 49 cols) and the 49 transposed k-tiles
     of A, delivered by 8 chunked DMAs so the PE consumes tiles while
     later chunks are still in flight. 49 matmuls accumulate into one
     fp32 PSUM bank; the result is DMA'd straight from PSUM to DRAM.
  4. Host sums the 8 partial results (the "all-reduce"), then applies
     the inverse of the unique() mapping to emit the two [50, 2]
     diagrams.

Raw Bass (no Tile). Each DMA has its own semaphore (inc 16 on
completion); no DMA carries an embedded wait, standalone engine
wait_ge ops order everything else.
"""

import numpy as np
import ml_dtypes

import concourse.bass as bass
import concourse.mybir as mybir
from concourse.bass_utils import run_bass_kernel_spmd

N_CORES = 8
P_FULL = 50000
H = W = 28
M = H * W  # 784
KT = 49  # k-subtiles of 128 per core
K_PER = KT * 128  # 6272; 8 * 6272 = 50176 >= 50000
K_PAD = N_CORES * K_PER

# Tiles per chunk: a small first chunk so the PE starts real tiles
# early (the PE clock ramps slowly and PE drain is the tail), big
# middle chunks to amortize per-DMA fixed costs, tapering at the end
# so the PE drains quickly once the last chunk's semaphore fires.
CHUNK_TILES = (4, 12, 12, 12, 8, 1)
assert sum(CHUNK_TILES) == KT

BF16 = ml_dtypes.bfloat16
F32 = np.float32


def build_nc(nr: int) -> bass.Bass:
    f32 = mybir.dt.float32
    bf16 = mybir.dt.bfloat16
    nc = bass.Bass("TRN2")
    ncols = KT + KT * nr  # p chunk cols, then 49 tiles of nr cols

    # Column ranges per chunk: chunk 0 also carries the p cols. Each
    # chunk gets its own DRAM tensor so the DMA source is fully
    # contiguous (a strided [128, x] read of one big tensor measured at
    # ~200 GB/s; contiguous blocks stream at full rate).
    bounds = []
    t0 = 0
    for g, gt in enumerate(CHUNK_TILES):
        c0 = 0 if g == 0 else KT + t0 * nr
        c1 = KT + (t0 + gt) * nr
        bounds.append((t0, t0 + gt, c0, c1))
        t0 += gt

    aw_ds = [
        nc.dram_tensor(f"aw{g}", [128, c1 - c0], bf16, kind="ExternalInput")
        for g, (_, _, c0, c1) in enumerate(bounds)
    ]
    out_d = nc.dram_tensor("outp", [1, 256], f32, kind="ExternalOutput")

    from contextlib import ExitStack

    OUT_PAD = 256  # staging padded so the out DMA is one clean descriptor

    with ExitStack() as stk:
        a_sb = stk.enter_context(nc.sbuf_tensor("a_sb", [128, ncols], bf16))
        o_sb = stk.enter_context(nc.sbuf_tensor("o_sb", [1, OUT_PAD], f32))
        warm_sb = stk.enter_context(nc.sbuf_tensor("warm_sb", [128, 257], bf16))
        ps = stk.enter_context(nc.psum_tensor("ps", [1, nr], f32))
        ps_w = stk.enter_context(nc.psum_tensor("ps_w", [1, 256], f32))
        ch_sems = [
            stk.enter_context(nc.semaphore(f"ch{g}"))
            for g in range(len(CHUNK_TILES))
        ]
        pe_sem = stk.enter_context(nc.semaphore("pe_sem"))
        dve_sem = stk.enter_context(nc.semaphore("dve_sem"))
        out_sem = stk.enter_context(nc.semaphore("out_sem"))
        block = stk.enter_context(nc.Block(no_gpsimd_drain=True))

        @block.sync
        def _(sync):
            # Chunks split across two HWDGE queues (SP + Activation):
            # transfers start on both immediately, and one queue's
            # chunk-boundary bubbles are covered by the other's
            # in-flight work.
            for g, (_, _, c0, c1) in enumerate(bounds):
                if g % 2 == 0:
                    sync.dma_start(a_sb[:, c0:c1], aw_ds[g][:, :]).then_inc(
                        ch_sems[g], 16
                    )
            # Pre-armed output DMA: issued during the stream with an
            # embedded wait on dve_sem, so the transfer fires the moment
            # the DVE eviction lands -- no SP wake + issue + DGE latency
            # in the tail. No completion wait either: SP's block-exit
            # drain holds the exit barrier until the queue is empty, and
            # the host readback is milliseconds later still.
            sync.dma_start(out_d[:, :], o_sb[:, :]).then_inc(
                out_sem, 16
            ).wait_op(dve_sem, 1, "sem-ge")

        @block.scalar
        def _(scalar):
            for g, (_, _, c0, c1) in enumerate(bounds):
                if g % 2 == 1:
                    scalar.dma_start(a_sb[:, c0:c1], aw_ds[g][:, :]).then_inc(
                        ch_sems[g], 16
                    )

        @block.tensor
        def _(tensor):
            # Warm-up matmuls on scratch data: the PE clock ramps only
            # after ~5-6us of continuous activity, and real work is
            # gated on the first DMA chunk (~2.5us in). These bridge the
            # gap so the ramp clock starts at t=0; sized to end right as
            # chunk 0 lands (over-long warm-up delays real tiles).
            for w in range(5):
                nc.tensor.matmul(
                    ps_w[:, :],
                    warm_sb[:, w : w + 1],
                    warm_sb[:, 1:257],
                    start=True,
                    stop=True,
                )
            last = None
            for g, (ta, tb, _, _) in enumerate(bounds):
                tensor.wait_ge(ch_sems[g], 16)
                for t in range(ta, tb):
                    last = nc.tensor.matmul(
                        ps[:, :],
                        a_sb[:, t : t + 1],
                        a_sb[:, KT + t * nr : KT + (t + 1) * nr],
                        start=(t == 0),
                        stop=(t == KT - 1),
                    )
            last.then_inc(pe_sem, 1)

        @block.vector
        def _(vector):
            vector.wait_ge(pe_sem, 1)
            nc.vector.tensor_copy(o_sb[:, :nr], ps[:, :]).then_inc(dve_sem, 1)

    return nc


_NC_CACHE: dict[int, bass.Bass] = {}


def get_nc(nr: int) -> bass.Bass:
    if nr not in _NC_CACHE:
        _NC_CACHE[nr] = build_nc(nr)
    return _NC_CACHE[nr]


def shard_inputs(A: np.ndarray, p: np.ndarray) -> list[dict]:
    """A: [NR, 50000] f32, p: [50000] f32 -> 8 per-core input maps."""
    nr = A.shape[0]
    Ab = np.zeros((nr, K_PAD), dtype=BF16)
    Ab[:, :P_FULL] = A.astype(BF16)
    pb = np.zeros(K_PAD, dtype=BF16)
    pb[:P_FULL] = p.astype(BF16)

    bounds = []
    t0 = 0
    for gt in CHUNK_TILES:
        bounds.append((t0, t0 + gt))
        t0 += gt

    in_maps = []
    for c in range(N_CORES):
        k0 = c * K_PER
        pw = np.ascontiguousarray(pb[k0 : k0 + K_PER].reshape(KT, 128).T)
        tiles = (
            Ab[:, k0 : k0 + K_PER]
            .T.reshape(KT, 128, nr)
            .transpose(1, 0, 2)
            .reshape(128, KT * nr)
        )
        im = {}
        for g, (ta, tb) in enumerate(bounds):
            part = tiles[:, ta * nr : tb * nr]
            if g == 0:
                part = np.concatenate([pw, part], axis=1)
            im[f"aw{g}"] = np.ascontiguousarray(part)
        in_maps.append(im)
    return in_maps


def run(p, I, J, inds1, inds2, trace=False, **run_kwargs):
    """Returns ((dgm1, dgm2), BassKernelResults)."""
    p = np.asarray(p, dtype=F32)
    I = np.asarray(I, dtype=F32)
    J = np.asarray(J, dtype=F32)
    inds1 = np.asarray(inds1)
    inds2 = np.asarray(inds2)

    rows1 = inds1[:, 0].astype(np.int64) * W + inds1[:, 1].astype(np.int64)
    rows2 = inds2[:, 0].astype(np.int64) * W + inds2[:, 1].astype(np.int64)
    u1, inv1 = np.unique(rows1, return_inverse=True)
    u2, inv2 = np.unique(rows2, return_inverse=True)
    n1 = len(u1)

    A = np.concatenate([I[u1], J[u2]], axis=0)
    nr = A.shape[0]

    in_maps = shard_inputs(A, p)
    nc = get_nc(nr)
    res = run_bass_kernel_spmd(
        nc, in_maps, list(range(N_CORES)), trace=trace, **run_kwargs
    )
    tot = np.zeros(nr, dtype=np.float64)
    for r in res.results:
        tot += r["outp"][0, :nr].astype(np.float64)
    dgm1 = tot[:n1][inv1].reshape(-1, 2).astype(F32)
    dgm2 = tot[n1:][inv2].reshape(-1, 2).astype(F32)
    return (dgm1, dgm2), res


def kernel(p, I, J, inds1, inds2):
    out, _ = run(p, I, J, inds1, inds2, trace=False)
    return out


# revision 23
# speedup vs baseline: 1.1320x; 1.1320x over previous
"""Distributed gathered-row matvec kernel for nn_CubicalModel_ISM.

Reference computes Xp = I @ p, Yp = J @ p (I, J: [784, 50000]) and then
gathers 100 (with repeats) elements from each 28x28 reshape. Only the
gathered rows matter, so the kernel:

  1. Host: dedupes the gather rows -> u1 (rows of I), u2 (rows of J),
     NR = |u1| + |u2| (~188 of the 1568 total rows). Builds
     A = concat(I[u1], J[u2]) : [NR, 50000] and computes only A @ p.
  2. Rounds A and p to bf16 (single plane). The bf16 quantization error
     of a 50k-term dot product concentrates around 3e-3 relative --
     far inside the 2e-2 gate -- while halving HBM traffic.
  3. Shards the contraction dim across 8 cores (6272 = 49*128 per core,
     zero padded). Per core a single DRAM stream [128, 49 + 49*NR] bf16
     carries the p chunk (first 49 cols) and the 49 transposed k-tiles
     of A, delivered by 8 chunked DMAs so the PE consumes tiles while
     later chunks are still in flight. 49 matmuls accumulate into one
     fp32 PSUM bank; the result is DMA'd straight from PSUM to DRAM.
  4. Host sums the 8 partial results (the "all-reduce"), then applies
     the inverse of the unique() mapping to emit the two [50, 2]
     diagrams.

Raw Bass (no Tile). Each DMA has its own semaphore (inc 16 on
completion); no DMA carries an embedded wait, standalone engine
wait_ge ops order everything else.
"""

import numpy as np
import ml_dtypes

import concourse.bass as bass
import concourse.mybir as mybir
from concourse.bass_utils import run_bass_kernel_spmd

N_CORES = 8
P_FULL = 50000
H = W = 28
M = H * W  # 784
KT = 49  # k-subtiles of 128 per core
K_PER = KT * 128  # 6272; 8 * 6272 = 50176 >= 50000
K_PAD = N_CORES * K_PER

# Tiles per chunk: a small first chunk so the PE starts real tiles
# early (the PE clock ramps slowly and PE drain is the tail), big
# middle chunks to amortize per-DMA fixed costs, tapering at the end
# so the PE drains quickly once the last chunk's semaphore fires.
CHUNK_TILES = (4, 12, 12, 12, 8, 1)
assert sum(CHUNK_TILES) == KT

BF16 = ml_dtypes.bfloat16
F32 = np.float32


def build_nc(nr: int) -> bass.Bass:
    f32 = mybir.dt.float32
    bf16 = mybir.dt.bfloat16
    nc = bass.Bass("TRN2")
    ncols = KT + KT * nr  # p chunk cols, then 49 tiles of nr cols

    # Column ranges per chunk: chunk 0 also carries the p cols. Each
    # chunk gets its own DRAM tensor so the DMA source is fully
    # contiguous (a strided [128, x] read of one big tensor measured at
    # ~200 GB/s; contiguous blocks stream at full rate).
    bounds = []
    t0 = 0
    for g, gt in enumerate(CHUNK_TILES):
        c0 = 0 if g == 0 else KT + t0 * nr
        c1 = KT + (t0 + gt) * nr
        bounds.append((t0, t0 + gt, c0, c1))
        t0 += gt

    aw_ds = [
        nc.dram_tensor(f"aw{g}", [128, c1 - c0], bf16, kind="ExternalInput")
        for g, (_, _, c0, c1) in enumerate(bounds)
    ]
    out_d = nc.dram_tensor("outp", [1, 256], f32, kind="ExternalOutput")

    from contextlib import ExitStack

    OUT_PAD = 256  # staging padded so the out DMA is one clean descriptor

    with ExitStack() as stk:
        a_sb = stk.enter_context(nc.sbuf_tensor("a_sb", [128, ncols], bf16))
        o_sb = stk.enter_context(nc.sbuf_tensor("o_sb", [1, OUT_PAD], f32))
        warm_sb = stk.enter_context(nc.sbuf_tensor("warm_sb", [128, 257], bf16))
        ps = stk.enter_context(nc.psum_tensor("ps", [1, nr], f32))
        ps_w = stk.enter_context(nc.psum_tensor("ps_w", [1, 256], f32))
        ch_sems = [
            stk.enter_context(nc.semaphore(f"ch{g}"))
            for g in range(len(CHUNK_TILES))
        ]
        pe_sem = stk.enter_context(nc.semaphore("pe_sem"))
        dve_sem = stk.enter_context(nc.semaphore("dve_sem"))
        out_sem = stk.enter_context(nc.semaphore("out_sem"))
        block = stk.enter_context(nc.Block(no_gpsimd_drain=True))

        @block.sync
        def _(sync):
            # Chunks split across two HWDGE queues (SP + Activation):
            # transfers start on both immediately, and one queue's
            # chunk-boundary bubbles are covered by the other's
            # in-flight work.
            for g, (_, _, c0, c1) in enumerate(bounds):
                if g % 2 == 0:
                    sync.dma_start(a_sb[:, c0:c1], aw_ds[g][:, :]).then_inc(
                        ch_sems[g], 16
                    )
            # Pre-armed output DMA: issued during the stream with an
            # embedded wait on dve_sem, so the transfer fires the moment
            # the DVE eviction lands -- no SP wake + issue + DGE latency
            # in the tail. No completion wait either: SP's block-exit
            # drain holds the exit barrier until the queue is empty, and
            # the host readback is milliseconds later still.
            sync.dma_start(out_d[:, :], o_sb[:, :]).then_inc(
                out_sem, 16
            ).wait_op(dve_sem, 1, "sem-ge")

        @block.scalar
        def _(scalar):
            for g, (_, _, c0, c1) in enumerate(bounds):
                if g % 2 == 1:
                    scalar.dma_start(a_sb[:, c0:c1], aw_ds[g][:, :]).then_inc(
                        ch_sems[g], 16
                    )

        @block.tensor
        def _(tensor):
            # Warm-up matmuls on scratch data: the PE clock ramps only
            # after ~5-6us of continuous activity, and real work is
            # gated on the first DMA chunk (~2.5us in). These bridge the
            # gap so the ramp clock starts at t=0; sized to end right as
            # chunk 0 lands (over-long warm-up delays real tiles).
            for w in range(9):
                nc.tensor.matmul(
                    ps_w[:, :],
                    warm_sb[:, w : w + 1],
                    warm_sb[:, 1:257],
                    start=True,
                    stop=True,
                )
            last = None
            for g, (ta, tb, _, _) in enumerate(bounds):
                tensor.wait_ge(ch_sems[g], 16)
                for t in range(ta, tb):
                    last = nc.tensor.matmul(
                        ps[:, :],
                        a_sb[:, t : t + 1],
                        a_sb[:, KT + t * nr : KT + (t + 1) * nr],
                        start=(t == 0),
                        stop=(t == KT - 1),
                    )
            last.then_inc(pe_sem, 1)

        @block.vector
        def _(vector):
            vector.wait_ge(pe_sem, 1)
            nc.vector.tensor_copy(o_sb[:, :nr], ps[:, :]).then_inc(dve_sem, 1)

    return nc


_NC_CACHE: dict[int, bass.Bass] = {}


def get_nc(nr: int) -> bass.Bass:
    if nr not in _NC_CACHE:
        _NC_CACHE[nr] = build_nc(nr)
    return _NC_CACHE[nr]


def shard_inputs(A: np.ndarray, p: np.ndarray) -> list[dict]:
    """A: [NR, 50000] f32, p: [50000] f32 -> 8 per-core input maps."""
    nr = A.shape[0]
    Ab = np.zeros((nr, K_PAD), dtype=BF16)
    Ab[:, :P_FULL] = A.astype(BF16)
    pb = np.zeros(K_PAD, dtype=BF16)
    pb[:P_FULL] = p.astype(BF16)

    bounds = []
    t0 = 0
    for gt in CHUNK_TILES:
        bounds.append((t0, t0 + gt))
        t0 += gt

    in_maps = []
    for c in range(N_CORES):
        k0 = c * K_PER
        pw = np.ascontiguousarray(pb[k0 : k0 + K_PER].reshape(KT, 128).T)
        tiles = (
            Ab[:, k0 : k0 + K_PER]
            .T.reshape(KT, 128, nr)
            .transpose(1, 0, 2)
            .reshape(128, KT * nr)
        )
        im = {}
        for g, (ta, tb) in enumerate(bounds):
            part = tiles[:, ta * nr : tb * nr]
            if g == 0:
                part = np.concatenate([pw, part], axis=1)
            im[f"aw{g}"] = np.ascontiguousarray(part)
        in_maps.append(im)
    return in_maps


def run(p, I, J, inds1, inds2, trace=False, **run_kwargs):
    """Returns ((dgm1, dgm2), BassKernelResults)."""
    p = np.asarray(p, dtype=F32)
    I = np.asarray(I, dtype=F32)
    J = np.asarray(J, dtype=F32)
    inds1 = np.asarray(inds1)
    inds2 = np.asarray(inds2)

    rows1 = inds1[:, 0].astype(np.int64) * W + inds1[:, 1].astype(np.int64)
    rows2 = inds2[:, 0].astype(np.int64) * W + inds2[:, 1].astype(np.int64)
    u1, inv1 = np.unique(rows1, return_inverse=True)
    u2, inv2 = np.unique(rows2, return_inverse=True)
    n1 = len(u1)

    A = np.concatenate([I[u1], J[u2]], axis=0)
    nr = A.shape[0]

    in_maps = shard_inputs(A, p)
    nc = get_nc(nr)
    res = run_bass_kernel_spmd(
        nc, in_maps, list(range(N_CORES)), trace=trace, **run_kwargs
    )
    tot = np.zeros(nr, dtype=np.float64)
    for r in res.results:
        tot += r["outp"][0, :nr].astype(np.float64)
    dgm1 = tot[:n1][inv1].reshape(-1, 2).astype(F32)
    dgm2 = tot[n1:][inv2].reshape(-1, 2).astype(F32)
    return (dgm1, dgm2), res


def kernel(p, I, J, inds1, inds2):
    out, _ = run(p, I, J, inds1, inds2, trace=False)
    return out


# revision 25
# speedup vs baseline: 1.1688x; 1.0324x over previous
"""Distributed gathered-row matvec kernel for nn_CubicalModel_ISM.

Reference computes Xp = I @ p, Yp = J @ p (I, J: [784, 50000]) and then
gathers 100 (with repeats) elements from each 28x28 reshape. Only the
gathered rows matter, so the kernel:

  1. Host: dedupes the gather rows -> u1 (rows of I), u2 (rows of J),
     NR = |u1| + |u2| (~188 of the 1568 total rows). Builds
     A = concat(I[u1], J[u2]) : [NR, 50000] and computes only A @ p.
  2. Rounds A and p to bf16 (single plane). The bf16 quantization error
     of a 50k-term dot product concentrates around 3e-3 relative --
     far inside the 2e-2 gate -- while halving HBM traffic.
  3. Shards the contraction dim across 8 cores (6272 = 49*128 per core,
     zero padded). Per core a single DRAM stream [128, 49 + 49*NR] bf16
     carries the p chunk (first 49 cols) and the 49 transposed k-tiles
     of A, delivered by 8 chunked DMAs so the PE consumes tiles while
     later chunks are still in flight. 49 matmuls accumulate into one
     fp32 PSUM bank; the result is DMA'd straight from PSUM to DRAM.
  4. Host sums the 8 partial results (the "all-reduce"), then applies
     the inverse of the unique() mapping to emit the two [50, 2]
     diagrams.

Raw Bass (no Tile). Each DMA has its own semaphore (inc 16 on
completion); no DMA carries an embedded wait, standalone engine
wait_ge ops order everything else.
"""

import numpy as np
import ml_dtypes

import concourse.bass as bass
import concourse.mybir as mybir
from concourse.bass_utils import run_bass_kernel_spmd

N_CORES = 8
P_FULL = 50000
H = W = 28
M = H * W  # 784
KT = 49  # k-subtiles of 128 per core
K_PER = KT * 128  # 6272; 8 * 6272 = 50176 >= 50000
K_PAD = N_CORES * K_PER

# Tiles per chunk: a small first chunk so the PE starts real tiles
# early (the PE clock ramps slowly and PE drain is the tail), big
# middle chunks to amortize per-DMA fixed costs, tapering at the end
# so the PE drains quickly once the last chunk's semaphore fires.
import os as _os

_CFG = _os.environ.get("BASS_CFG", "C2")
if _CFG == "C1":
    CHUNK_TILES = (13, 12, 12, 8, 3, 1)
    WARMUP_MM = 16
else:
    CHUNK_TILES = (4, 12, 12, 12, 8, 1)
    WARMUP_MM = 9
assert sum(CHUNK_TILES) == KT

BF16 = ml_dtypes.bfloat16
F32 = np.float32


def build_nc(nr: int) -> bass.Bass:
    f32 = mybir.dt.float32
    bf16 = mybir.dt.bfloat16
    nc = bass.Bass("TRN2")
    ncols = KT + KT * nr  # p chunk cols, then 49 tiles of nr cols

    # Column ranges per chunk: chunk 0 also carries the p cols. Each
    # chunk gets its own DRAM tensor so the DMA source is fully
    # contiguous (a strided [128, x] read of one big tensor measured at
    # ~200 GB/s; contiguous blocks stream at full rate).
    bounds = []
    t0 = 0
    for g, gt in enumerate(CHUNK_TILES):
        c0 = 0 if g == 0 else KT + t0 * nr
        c1 = KT + (t0 + gt) * nr
        bounds.append((t0, t0 + gt, c0, c1))
        t0 += gt

    aw_ds = [
        nc.dram_tensor(f"aw{g}", [128, c1 - c0], bf16, kind="ExternalInput")
        for g, (_, _, c0, c1) in enumerate(bounds)
    ]
    out_d = nc.dram_tensor("outp", [1, 256], f32, kind="ExternalOutput")

    from contextlib import ExitStack

    OUT_PAD = 256  # staging padded so the out DMA is one clean descriptor

    with ExitStack() as stk:
        a_sb = stk.enter_context(nc.sbuf_tensor("a_sb", [128, ncols], bf16))
        o_sb = stk.enter_context(nc.sbuf_tensor("o_sb", [1, OUT_PAD], f32))
        warm_sb = stk.enter_context(nc.sbuf_tensor("warm_sb", [128, 257], bf16))
        ps = stk.enter_context(nc.psum_tensor("ps", [1, nr], f32))
        ps_w = stk.enter_context(nc.psum_tensor("ps_w", [1, 256], f32))
        ch_sems = [
            stk.enter_context(nc.semaphore(f"ch{g}"))
            for g in range(len(CHUNK_TILES))
        ]
        pe_sem = stk.enter_context(nc.semaphore("pe_sem"))
        dve_sem = stk.enter_context(nc.semaphore("dve_sem"))
        out_sem = stk.enter_context(nc.semaphore("out_sem"))
        block = stk.enter_context(nc.Block(no_gpsimd_drain=True))

        @block.sync
        def _(sync):
            # Chunks split across two HWDGE queues (SP + Activation):
            # transfers start on both immediately, and one queue's
            # chunk-boundary bubbles are covered by the other's
            # in-flight work.
            for g, (_, _, c0, c1) in enumerate(bounds):
                if g % 2 == 0:
                    sync.dma_start(a_sb[:, c0:c1], aw_ds[g][:, :]).then_inc(
                        ch_sems[g], 16
                    )
            # Pre-armed output DMA: issued during the stream with an
            # embedded wait on dve_sem, so the transfer fires the moment
            # the DVE eviction lands -- no SP wake + issue + DGE latency
            # in the tail. No completion wait either: SP's block-exit
            # drain holds the exit barrier until the queue is empty, and
            # the host readback is milliseconds later still.
            sync.dma_start(out_d[:, :], o_sb[:, :]).then_inc(
                out_sem, 16
            ).wait_op(dve_sem, 1, "sem-ge")

        @block.scalar
        def _(scalar):
            for g, (_, _, c0, c1) in enumerate(bounds):
                if g % 2 == 1:
                    scalar.dma_start(a_sb[:, c0:c1], aw_ds[g][:, :]).then_inc(
                        ch_sems[g], 16
                    )

        @block.tensor
        def _(tensor):
            # Warm-up matmuls on scratch data: the PE clock ramps only
            # after ~5-6us of continuous activity, and real work is
            # gated on the first DMA chunk (~2.5us in). These bridge the
            # gap so the ramp clock starts at t=0; sized to end right as
            # chunk 0 lands (over-long warm-up delays real tiles).
            for w in range(WARMUP_MM):
                nc.tensor.matmul(
                    ps_w[:, :],
                    warm_sb[:, w : w + 1],
                    warm_sb[:, 1:257],
                    start=True,
                    stop=True,
                )
            last = None
            for g, (ta, tb, _, _) in enumerate(bounds):
                tensor.wait_ge(ch_sems[g], 16)
                for t in range(ta, tb):
                    last = nc.tensor.matmul(
                        ps[:, :],
                        a_sb[:, t : t + 1],
                        a_sb[:, KT + t * nr : KT + (t + 1) * nr],
                        start=(t == 0),
                        stop=(t == KT - 1),
                    )
            last.then_inc(pe_sem, 1)

        @block.vector
        def _(vector):
            vector.wait_ge(pe_sem, 1)
            nc.vector.tensor_copy(o_sb[:, :nr], ps[:, :]).then_inc(dve_sem, 1)

    return nc


_NC_CACHE: dict[int, bass.Bass] = {}


def get_nc(nr: int) -> bass.Bass:
    if nr not in _NC_CACHE:
        _NC_CACHE[nr] = build_nc(nr)
    return _NC_CACHE[nr]


def shard_inputs(A: np.ndarray, p: np.ndarray) -> list[dict]:
    """A: [NR, 50000] f32, p: [50000] f32 -> 8 per-core input maps."""
    nr = A.shape[0]
    Ab = np.zeros((nr, K_PAD), dtype=BF16)
    Ab[:, :P_FULL] = A.astype(BF16)
    pb = np.zeros(K_PAD, dtype=BF16)
    pb[:P_FULL] = p.astype(BF16)

    bounds = []
    t0 = 0
    for gt in CHUNK_TILES:
        bounds.append((t0, t0 + gt))
        t0 += gt

    in_maps = []
    for c in range(N_CORES):
        k0 = c * K_PER
        pw = np.ascontiguousarray(pb[k0 : k0 + K_PER].reshape(KT, 128).T)
        tiles = (
            Ab[:, k0 : k0 + K_PER]
            .T.reshape(KT, 128, nr)
            .transpose(1, 0, 2)
            .reshape(128, KT * nr)
        )
        im = {}
        for g, (ta, tb) in enumerate(bounds):
            part = tiles[:, ta * nr : tb * nr]
            if g == 0:
                part = np.concatenate([pw, part], axis=1)
            im[f"aw{g}"] = np.ascontiguousarray(part)
        in_maps.append(im)
    return in_maps


def run(p, I, J, inds1, inds2, trace=False, **run_kwargs):
    """Returns ((dgm1, dgm2), BassKernelResults)."""
    p = np.asarray(p, dtype=F32)
    I = np.asarray(I, dtype=F32)
    J = np.asarray(J, dtype=F32)
    inds1 = np.asarray(inds1)
    inds2 = np.asarray(inds2)

    rows1 = inds1[:, 0].astype(np.int64) * W + inds1[:, 1].astype(np.int64)
    rows2 = inds2[:, 0].astype(np.int64) * W + inds2[:, 1].astype(np.int64)
    u1, inv1 = np.unique(rows1, return_inverse=True)
    u2, inv2 = np.unique(rows2, return_inverse=True)
    n1 = len(u1)

    A = np.concatenate([I[u1], J[u2]], axis=0)
    nr = A.shape[0]

    in_maps = shard_inputs(A, p)
    nc = get_nc(nr)
    res = run_bass_kernel_spmd(
        nc, in_maps, list(range(N_CORES)), trace=trace, **run_kwargs
    )
    tot = np.zeros(nr, dtype=np.float64)
    for r in res.results:
        tot += r["outp"][0, :nr].astype(np.float64)
    dgm1 = tot[:n1][inv1].reshape(-1, 2).astype(F32)
    dgm2 = tot[n1:][inv2].reshape(-1, 2).astype(F32)
    return (dgm1, dgm2), res


def kernel(p, I, J, inds1, inds2):
    out, _ = run(p, I, J, inds1, inds2, trace=False)
    return out
